# revision 2
# baseline (speedup 1.0000x reference)
"""EfficientAttention Trainium2 Bass kernel.

Reference computation (per token t, H=16 heads, hd=64):
  Q = x @ Wq.T ; K = x @ Wk.T ; V = x @ Wv.T        (d = 1024)
  sK = softmax over heads of K^T      : sK[d,h] = expK[h,d] / rk[d]
  tran_V = sK @ V                      (64 x 64)
  out = softmax(Q, axis=-1) @ tran_V   (16 x 64) -> flatten 1024

Equivalent form used here (per token):
  A[h',h]  = sum_d expK[h',d] * sQ'[h,d]   with sQ' = expQ * rqi[h] * rki[d]
  out[h,:] = sum_h' A[h',h] * V[h',:]

Sharding: data-parallel over the 16384 tokens across 8 cores (2048 each).
Weights replicated; x is pre-transposed to [feature, token] on the host so
no on-device x transposes are needed. W_V columns are permuted host-side so
V arrives as [t, (e, h')] (e-outer), making mm2 a contiguous-stream DVE op.

Device layout: tokens on SBUF partitions (128-token tiles).
  PE   : the three 1024x1024 projections (bf16) + slab extraction
         transposes + per-token mm1 + A-back transposes
  ACT  : psum->sbuf evictions fused with exp for Q,K
  DVE  : softmax normalizers; mm2 as 16 broadcast-muls + one segmented
         reduce (bf16 in, fp16 out)
"""

import numpy as np
import ml_dtypes
from contextlib import ExitStack

import concourse.bass as bass
import concourse.mybir as mybir
import concourse.tile as tile
from concourse import bacc
from concourse.bass_utils import run_bass_kernel_spmd

DIMS = 1024
HEADS = 16
HD = 64
N_CORES = 8
B, L = 4, 4096
TOKENS = B * L
TOK_PER_CORE = TOKENS // N_CORES  # 2048
P = 128                           # tokens per tile (SBUF partitions)
N_TILES = TOK_PER_CORE // P       # 16

FP32 = mybir.dt.float32
FP16 = mybir.dt.float16
BF16 = mybir.dt.bfloat16

_COMPILED = {}


def _build_kernel():
    nc = bacc.Bacc("TRN2", target_bir_lowering=False)

    # x pre-transposed on host: [feature j, token t]
    xT_in = nc.dram_tensor("xT", [DIMS, TOK_PER_CORE], BF16, kind="ExternalInput")
    wq_in = nc.dram_tensor("wq", [DIMS, DIMS], BF16, kind="ExternalInput")
    wk_in = nc.dram_tensor("wk", [DIMS, DIMS], BF16, kind="ExternalInput")
    wv_in = nc.dram_tensor("wv", [DIMS, DIMS], BF16, kind="ExternalInput")
    ident_in = nc.dram_tensor("ident", [P, P], BF16, kind="ExternalInput")
    out_d = nc.dram_tensor("out", [TOK_PER_CORE, DIMS], FP16, kind="ExternalOutput")

    with tile.TileContext(nc) as tc, ExitStack() as ctx:
        consts = ctx.enter_context(tc.tile_pool(name="consts", bufs=1))
        wpool = ctx.enter_context(tc.tile_pool(name="weights", bufs=1))
        xpool = ctx.enter_context(tc.tile_pool(name="x", bufs=4))
        smpool = ctx.enter_context(tc.tile_pool(name="sm", bufs=2))
        slabpool = ctx.enter_context(tc.tile_pool(name="slab", bufs=2))
        mmpool = ctx.enter_context(tc.tile_pool(name="mm", bufs=2))
        prodpool = ctx.enter_context(tc.tile_pool(name="prod", bufs=1))
        opool = ctx.enter_context(tc.tile_pool(name="outs", bufs=2))
        # PSUM budget (8 banks): pp(proj)=2, tp(extract slabs)=2x2, pa/ab=2
        ps_pp = ctx.enter_context(tc.tile_pool(name="ps_pp", bufs=2, space="PSUM"))
        ps_tp = ctx.enter_context(tc.tile_pool(name="ps_tp", bufs=4, space="PSUM"))
        ps_pa = ctx.enter_context(tc.tile_pool(name="ps_pa", bufs=2, space="PSUM"))

        ident = consts.tile([P, P], BF16)
        nc.sync.dma_start(ident[:], ident_in[:])

        ws = {}
        for name, w_in in (("q", wq_in), ("k", wk_in), ("v", wv_in)):
            w = wpool.tile([P, 8 * DIMS], BF16, tag=f"w{name}")
            for c in range(8):
                nc.sync.dma_start(w[:, c * DIMS:(c + 1) * DIMS],
                                  w_in[c * P:(c + 1) * P, :])
            ws[name] = w

        def load_xT(it):
            xt = xpool.tile([P, DIMS], BF16, tag="xt", name=f"xT{it}")
            src = xT_in[:, it * P:(it + 1) * P].rearrange("(c p) t -> p c t", p=P)
            nc.sync.dma_start(xt[:].rearrange("p (c t) -> p c t", t=P), src)
            return xt

        xts = {}
        for pre in range(2):
            xts[pre] = load_xT(pre)

        for it in range(N_TILES):
            # 1) x^T tile [128 j-chunk, (c, t)] bf16 (chunk c = features 128c..)
            xt = xts.pop(it) if it in xts else load_xT(it)
            if it + 2 < N_TILES and (it + 2) not in xts:
                xts[it + 2] = load_xT(it + 2)

            # 2) projections; per proj 2 psum banks, evicted via ACT
            #    (exp fused for Q,K; plain copy for V)
            expq = smpool.tile([P, DIMS], BF16, tag="expq")
            expk = smpool.tile([P, DIMS], BF16, tag="expk")
            vt = smpool.tile([P, DIMS], BF16, tag="vt")
            for pname, dst, func in (
                ("q", expq, mybir.ActivationFunctionType.Exp),
                ("k", expk, mybir.ActivationFunctionType.Exp),
                ("v", vt, None),
            ):
                w = ws[pname]
                pps = []
                for nb in range(2):
                    pp = ps_pp.tile([P, 512], FP32, tag="pp",
                                    name=f"pp{it}_{pname}{nb}")
                    pps.append(pp)
                # lhsT (x chunk) stays stationary across the two half-banks
                for c in range(8):
                    for nb in range(2):
                        nc.tensor.matmul(
                            pps[nb][:],
                            lhsT=xt[:, c * P:(c + 1) * P],
                            rhs=w[:, c * DIMS + nb * 512: c * DIMS + nb * 512 + 512],
                            start=(c == 0), stop=(c == 7),
                        )
                for nb in range(2):
                    sl = slice(nb * 512, nb * 512 + 512)
                    if func is None:
                        nc.scalar.copy(dst[:, sl], pps[nb][:])
                    else:
                        nc.scalar.activation(dst[:, sl], pps[nb][:], func)

            # 3) softmax normalizers on DVE
            rq = smpool.tile([P, HEADS], FP32, tag="rq")       # sum_d expQ[h,d]
            nc.vector.reduce_sum(rq[:], expq[:].rearrange("p (h d) -> p h d", d=HD),
                                 axis=mybir.AxisListType.X)
            # rk[d] = sum_h expK[t,(h,d)] via contiguous halving adds
            t1 = smpool.tile([P, 512], BF16, tag="t1")
            nc.vector.tensor_add(t1[:], expk[:, 0:512], expk[:, 512:1024])
            t2 = smpool.tile([P, 256], BF16, tag="t2")
            nc.vector.tensor_add(t2[:], t1[:, 0:256], t1[:, 256:512])
            t3 = smpool.tile([P, 128], BF16, tag="t3")
            nc.vector.tensor_add(t3[:], t2[:, 0:128], t2[:, 128:256])
            rk = smpool.tile([P, HD], FP32, tag="rk")
            nc.vector.tensor_add(rk[:], t3[:, 0:HD], t3[:, HD:128])
            rqi = smpool.tile([P, HEADS], FP32, tag="rqi")
            nc.vector.reciprocal_approx_fast(rqi[:], rq[:])
            rki = smpool.tile([P, HD], FP32, tag="rki")
            nc.vector.reciprocal_approx_fast(rki[:], rk[:])
            rkib = smpool.tile([P, HD], BF16, tag="rkib")
            nc.scalar.copy(rkib[:], rki[:])
            rqib = smpool.tile([P, HEADS], BF16, tag="rqib")
            nc.scalar.copy(rqib[:], rqi[:])

            # 4) sQ'[t,(h,d)] = expQ * rki[d] * rqi[h]  (both softmax scales
            #    folded into the Q side; K side stays raw expK)
            sqt = smpool.tile([P, DIMS], BF16, tag="sqt")
            rkib_b = rkib[:].unsqueeze(1).broadcast_to([P, HEADS, HD])
            nc.vector.tensor_mul(sqt[:].rearrange("p (h d) -> p h d", d=HD),
                                 expq[:].rearrange("p (h d) -> p h d", d=HD),
                                 rkib_b)
            rqib_b = rqib[:].unsqueeze(2).broadcast_to([P, HEADS, HD])
            nc.vector.tensor_mul(sqt[:].rearrange("p (h d) -> p h d", d=HD),
                                 sqt[:].rearrange("p (h d) -> p h d", d=HD),
                                 rqib_b)

            # 5) extraction: per-head PE transposes -> feature-on-partition
            #    slabs QS/KS [64 d, 16 heads x 128 tokens] bf16
            slabs = {}
            for sname, srct in (("qs", sqt), ("ks", expk)):
                slab = slabpool.tile([HD, HEADS * P], BF16, tag=sname)
                for b in range(2):
                    ep = ps_tp.tile([HD, 8 * P], BF16, tag="tp",
                                    name=f"ep{it}_{sname}{b}")
                    for hh in range(8):
                        h = 8 * b + hh
                        nc.tensor.transpose(
                            ep[:, hh * P:(hh + 1) * P],
                            srct[:, h * HD:(h + 1) * HD],
                            ident[:])
                    nc.scalar.copy(slab[:, b * 8 * P:(b + 1) * 8 * P], ep[:])
                slabs[sname] = slab

            # 6) mm1 on PE: per token A[h',h] = sum_d KS[d,h'] * QS[d,h]
            #    out [16,16] per token, 32 tokens per psum bank, evicted bf16
            aev = mmpool.tile([HEADS, 2048], BF16, tag="aev")
            for bk in range(4):
                pa = ps_pa.tile([HEADS, 512], FP32, tag="pa",
                                name=f"pa{it}_{bk}")
                for ts in range(32):
                    t = 32 * bk + ts
                    nc.tensor.matmul(
                        pa[:, ts * HEADS:(ts + 1) * HEADS],
                        lhsT=slabs["ks"][:, t::P], rhs=slabs["qs"][:, t::P],
                        start=True, stop=True)
                nc.scalar.copy(aev[:, bk * 512:(bk + 1) * 512], pa[:])

            # 7) A back to token-partitions: per h transpose
            #    [16 h', 128 t] -> [128 t, 16 h'], pack into at [128, 256]
            at = mmpool.tile([P, HEADS * HEADS], BF16, tag="at")
            ab = ps_pa.tile([P, HEADS * HEADS], BF16, tag="pa",
                            name=f"ab{it}")
            for h in range(HEADS):
                nc.tensor.transpose(
                    ab[:, h * HEADS:(h + 1) * HEADS],
                    aev[:, h::HEADS], ident[0:HEADS, 0:HEADS])
            nc.scalar.copy(at[:], ab[:])

            # 8) mm2 on DVE: prod[t,(h,e,h')] = V[t,(e,h')] * A[t,(h,h')]
            #    then one segmented reduce over h' -> out[t,(h,e)]
            prod = prodpool.tile([P, HEADS * DIMS], BF16, tag="prod")
            v_view = vt[:].rearrange("p (e k) -> p e k", k=HEADS)
            for h in range(HEADS):
                a_b = at[:, h * HEADS:(h + 1) * HEADS] \
                    .unsqueeze(1).broadcast_to([P, HD, HEADS])
                nc.vector.tensor_mul(
                    prod[:, h * DIMS:(h + 1) * DIMS]
                        .rearrange("p (e k) -> p e k", k=HEADS),
                    v_view, a_b)
            ot = opool.tile([P, DIMS], FP16, tag="ot")
            with nc.allow_low_precision(reason="fp16 final output"):
                nc.vector.reduce_sum(
                    ot[:],
                    prod[:].rearrange("p (f k) -> p f k", k=HEADS),
                    axis=mybir.AxisListType.X)
            # 9) store
            nc.sync.dma_start(out_d[it * P:(it + 1) * P, :], ot[:])

    nc.compile()
    return nc


def kernel(input_seq_embs, W_Q, W_K, W_V):
    x = np.asarray(input_seq_embs, dtype=np.float32).reshape(TOKENS, DIMS)
    xT = np.ascontiguousarray(x.T).astype(ml_dtypes.bfloat16)  # [j, t]
    # torch Linear computes x @ W.T; our matmul wants rhs = W.T laid out
    # [contraction j, out i].
    wq = np.ascontiguousarray(np.asarray(W_Q, np.float32).T).astype(ml_dtypes.bfloat16)
    wk = np.ascontiguousarray(np.asarray(W_K, np.float32).T).astype(ml_dtypes.bfloat16)
    # W_V columns permuted so V features arrive e-outer: f' = e*16 + h'
    wvT = np.asarray(W_V, np.float32).T  # [j, (h', e)]
    wv = np.ascontiguousarray(
        wvT.reshape(DIMS, HEADS, HD).transpose(0, 2, 1).reshape(DIMS, DIMS)
    ).astype(ml_dtypes.bfloat16)
    ident = np.eye(P, dtype=ml_dtypes.bfloat16)

    if "nc" not in _COMPILED:
        _COMPILED["nc"] = _build_kernel()
    nc = _COMPILED["nc"]

    in_maps = []
    for c in range(N_CORES):
        shard = np.ascontiguousarray(xT[:, c * TOK_PER_CORE:(c + 1) * TOK_PER_CORE])
        in_maps.append({"xT": shard, "wq": wq, "wk": wk, "wv": wv, "ident": ident})

    import os
    trace = bool(int(os.environ.get("KERNEL_PROFILE", "0")))
    kw = {}
    if trace:
        kw = dict(trace=True, tmpdir=os.environ.get("KERNEL_TRACE_DIR") or None)
    res = run_bass_kernel_spmd(nc, in_maps, list(range(N_CORES)), **kw)
    if trace:
        print(f"HW exec time: {res.exec_time_ns} ns")
        _COMPILED["last_result"] = res
    outs = [np.asarray(res.results[c]["out"], dtype=np.float32)
            for c in range(N_CORES)]
    return np.concatenate(outs, axis=0).reshape(B, L, DIMS)


# revision 5
# speedup vs baseline: 1.4821x; 1.4821x over previous
"""EfficientAttention Trainium2 Bass kernel.

Reference computation (per token t, H=16 heads, hd=64):
  Q = x @ Wq.T ; K = x @ Wk.T ; V = x @ Wv.T        (d = 1024)
  sK = softmax over heads of K^T      : sK[d,h] = expK[h,d] / rk[d]
  tran_V = sK @ V                      (64 x 64)
  out = softmax(Q, axis=-1) @ tran_V   (16 x 64) -> flatten 1024

Equivalent form used here (per token):
  A[h',h]  = sum_d expK[h',d] * sQ'[h,d]   with sQ' = expQ * rki[d]
  out[h,:] = rqi[h] * sum_h' A[h',h] * V[h',:]

Sharding: data-parallel over the 16384 tokens across 8 cores (2048 each).
Weights replicated; x is pre-transposed to [feature, token] on the host so
no on-device x transposes are needed. W_V columns are permuted host-side so
V arrives as [t, (e, h')] (e-outer), making mm2 contiguous DVE streams.

Device layout: tokens on SBUF partitions (128-token tiles).
  PE   : three 1024x1024 projections (bf16, stationary reuse) + slab
         extraction transposes + per-token mm1
  ACT  : psum->sbuf evictions fused with exp for Q,K; strided A eviction
  DMA  : xbar transpose turns the A slab [16,(h,t)] into token-major
         at [128,(h,h')] in one instruction
  DVE  : softmax normalizers; mm2 as 4 width-4 broadcast-muls + pairwise
         adds (all 2x-eligible bf16 streams), fp16 out
"""

import numpy as np
import ml_dtypes
from contextlib import ExitStack

import concourse.bass as bass
import concourse.mybir as mybir
import concourse.tile as tile
from concourse import bacc
from concourse.bass_utils import run_bass_kernel_spmd

DIMS = 1024
HEADS = 16
HD = 64
N_CORES = 8
B, L = 4, 4096
TOKENS = B * L
TOK_PER_CORE = TOKENS // N_CORES  # 2048
P = 128                           # tokens per tile (SBUF partitions)
N_TILES = TOK_PER_CORE // P       # 16

FP32 = mybir.dt.float32
FP16 = mybir.dt.float16
BF16 = mybir.dt.bfloat16

_COMPILED = {}


def _build_kernel():
    nc = bacc.Bacc("TRN2", target_bir_lowering=False)

    # x pre-transposed on host: [feature j, token t]
    xT_in = nc.dram_tensor("xT", [DIMS, TOK_PER_CORE], BF16, kind="ExternalInput")
    wq_in = nc.dram_tensor("wq", [DIMS, DIMS], BF16, kind="ExternalInput")
    wk_in = nc.dram_tensor("wk", [DIMS, DIMS], BF16, kind="ExternalInput")
    wv_in = nc.dram_tensor("wv", [DIMS, DIMS], BF16, kind="ExternalInput")
    ident_in = nc.dram_tensor("ident", [P, P], BF16, kind="ExternalInput")
    out_d = nc.dram_tensor("out", [TOK_PER_CORE, DIMS], FP16, kind="ExternalOutput")

    with tile.TileContext(nc) as tc, ExitStack() as ctx:
        consts = ctx.enter_context(tc.tile_pool(name="consts", bufs=1))
        wpool = ctx.enter_context(tc.tile_pool(name="weights", bufs=1))
        xpool = ctx.enter_context(tc.tile_pool(name="x", bufs=4))
        smpool = ctx.enter_context(tc.tile_pool(name="sm", bufs=2))
        slabpool = ctx.enter_context(tc.tile_pool(name="slab", bufs=2))
        mmpool = ctx.enter_context(tc.tile_pool(name="mm", bufs=2))
        prodpool = ctx.enter_context(tc.tile_pool(name="prod", bufs=1))
        opool = ctx.enter_context(tc.tile_pool(name="outs", bufs=2))
        # PSUM budget (8 banks): pp(proj)=4, tp(extract slabs)=2, pa(mm1)=2
        ps_pp = ctx.enter_context(tc.tile_pool(name="ps_pp", bufs=4, space="PSUM"))
        ps_tp = ctx.enter_context(tc.tile_pool(name="ps_tp", bufs=2, space="PSUM"))
        ps_pa = ctx.enter_context(tc.tile_pool(name="ps_pa", bufs=2, space="PSUM"))

        ident = consts.tile([P, P], BF16)
        nc.sync.dma_start(ident[:], ident_in[:])

        ws = {}
        for name, w_in in (("q", wq_in), ("k", wk_in), ("v", wv_in)):
            w = wpool.tile([P, 8 * DIMS], BF16, tag=f"w{name}")
            for c in range(8):
                nc.sync.dma_start(w[:, c * DIMS:(c + 1) * DIMS],
                                  w_in[c * P:(c + 1) * P, :])
            ws[name] = w

        def load_xT(it):
            xt = xpool.tile([P, DIMS], BF16, tag="xt", name=f"xT{it}")
            src = xT_in[:, it * P:(it + 1) * P].rearrange("(c p) t -> p c t", p=P)
            nc.sync.dma_start(xt[:].rearrange("p (c t) -> p c t", t=P), src)
            return xt

        xts = {}
        for pre in range(2):
            xts[pre] = load_xT(pre)

        for it in range(N_TILES):
            # 1) x^T tile [128 j-chunk, (c, t)] bf16 (chunk c = features 128c..)
            xt = xts.pop(it) if it in xts else load_xT(it)
            if it + 2 < N_TILES and (it + 2) not in xts:
                xts[it + 2] = load_xT(it + 2)

            # 2) projections. q+k share the x stationary across 4 psum banks
            #    (2 half-banks each); v runs after. ACT evicts with fused exp.
            expq = smpool.tile([P, DIMS], BF16, tag="expq")
            expk = smpool.tile([P, DIMS], BF16, tag="expk")
            vt = smpool.tile([P, DIMS], BF16, tag="vt")
            pps = {}
            for pname in ("q", "k"):
                for nb in range(2):
                    pps[pname, nb] = ps_pp.tile(
                        [P, 512], FP32, tag="pp", name=f"pp{it}_{pname}{nb}")
            for c in range(8):
                for pname in ("q", "k"):
                    for nb in range(2):
                        nc.tensor.matmul(
                            pps[pname, nb][:],
                            lhsT=xt[:, c * P:(c + 1) * P],
                            rhs=ws[pname][:, c * DIMS + nb * 512:
                                          c * DIMS + nb * 512 + 512],
                            start=(c == 0), stop=(c == 7),
                        )
            for pname, dst in (("q", expq), ("k", expk)):
                for nb in range(2):
                    nc.scalar.activation(dst[:, nb * 512:nb * 512 + 512],
                                         pps[pname, nb][:],
                                         mybir.ActivationFunctionType.Exp)
            pps = {}
            for nb in range(2):
                pps[nb] = ps_pp.tile([P, 512], FP32, tag="pp",
                                     name=f"pp{it}_v{nb}")
            for c in range(8):
                for nb in range(2):
                    nc.tensor.matmul(
                        pps[nb][:],
                        lhsT=xt[:, c * P:(c + 1) * P],
                        rhs=ws["v"][:, c * DIMS + nb * 512:
                                    c * DIMS + nb * 512 + 512],
                        start=(c == 0), stop=(c == 7),
                    )
            for nb in range(2):
                nc.scalar.copy(vt[:, nb * 512:nb * 512 + 512], pps[nb][:])

            # 3) softmax normalizers on DVE
            rq = smpool.tile([P, HEADS], FP32, tag="rq")       # sum_d expQ[h,d]
            nc.vector.reduce_sum(rq[:], expq[:].rearrange("p (h d) -> p h d", d=HD),
                                 axis=mybir.AxisListType.X)
            # rk[d] = sum_h expK[t,(h,d)] via contiguous halving adds
            t1 = smpool.tile([P, 512], BF16, tag="t1")
            nc.vector.tensor_add(t1[:], expk[:, 0:512], expk[:, 512:1024])
            t2 = smpool.tile([P, 256], BF16, tag="t2")
            nc.vector.tensor_add(t2[:], t1[:, 0:256], t1[:, 256:512])
            t3 = smpool.tile([P, 128], BF16, tag="t3")
            nc.vector.tensor_add(t3[:], t2[:, 0:128], t2[:, 128:256])
            rk = smpool.tile([P, HD], FP32, tag="rk")
            nc.vector.tensor_add(rk[:], t3[:, 0:HD], t3[:, HD:128])
            rqi = smpool.tile([P, HEADS], FP32, tag="rqi")
            nc.vector.reciprocal_approx_fast(rqi[:], rq[:])
            rki = smpool.tile([P, HD], FP32, tag="rki")
            nc.vector.reciprocal_approx_fast(rki[:], rk[:])
            rkib = smpool.tile([P, HD], BF16, tag="rkib")
            nc.scalar.copy(rkib[:], rki[:])
            rqib = smpool.tile([P, HEADS], BF16, tag="rqib")
            nc.scalar.copy(rqib[:], rqi[:])

            # 4) sQ'[t,(h,d)] = expQ * rki[d]  (rqi folded into at below)
            sqt = smpool.tile([P, DIMS], BF16, tag="sqt")
            rkib_b = rkib[:].unsqueeze(1).broadcast_to([P, HEADS, HD])
            nc.vector.tensor_mul(sqt[:].rearrange("p (h d) -> p h d", d=HD),
                                 expq[:].rearrange("p (h d) -> p h d", d=HD),
                                 rkib_b)

            # 5) extraction: per-head PE transposes -> feature-on-partition
            #    slabs QS/KS [64 d, 16 heads x 128 tokens] bf16
            slabs = {}
            for sname, srct in (("qs", sqt), ("ks", expk)):
                slab = slabpool.tile([HD, HEADS * P], BF16, tag=sname)
                for b in range(2):
                    ep = ps_tp.tile([HD, 8 * P], BF16, tag="tp",
                                    name=f"ep{it}_{sname}{b}")
                    for hh in range(8):
                        h = 8 * b + hh
                        nc.tensor.transpose(
                            ep[:, hh * P:(hh + 1) * P],
                            srct[:, h * HD:(h + 1) * HD],
                            ident[:])
                    nc.scalar.copy(slab[:, b * 8 * P:(b + 1) * 8 * P], ep[:])
                slabs[sname] = slab

            # 6) mm1 on PE: per token A[h',h] = sum_d KS[d,h'] * QS[d,h]
            #    out [16,16] per token, 32 tokens per psum bank; evicted via
            #    ACT in (h, t) column order so the xbar transpose below lands
            #    token-major.
            aev = mmpool.tile([HEADS, 2048], BF16, tag="aev")  # [h', (h,t)]
            for bk in range(4):
                pa = ps_pa.tile([HEADS, 512], FP32, tag="pa",
                                name=f"pa{it}_{bk}")
                for ts in range(32):
                    t = 32 * bk + ts
                    nc.tensor.matmul(
                        pa[:, ts * HEADS:(ts + 1) * HEADS],
                        lhsT=slabs["ks"][:, t::P], rhs=slabs["qs"][:, t::P],
                        start=True, stop=True)
                # pa cols are (t_local, h); scatter to aev cols h*128 + bk*32 + t
                nc.scalar.copy(
                    aev[:].rearrange("p (h t) -> p h t", t=P)
                        [:, :, bk * 32:(bk + 1) * 32],
                    pa[:].rearrange("p (t h) -> p h t", h=HEADS))

            # 7) A to token-major via one DMA xbar transpose:
            #    at[t, (h,h')] = aev[h', h*128+t]; then fold rqi[h] in.
            at = mmpool.tile([P, HEADS * HEADS], BF16, tag="at")
            nc.sync.dma_start_transpose(
                at[:].rearrange("p (h k) -> p h k", k=HEADS), aev[:])
            rqib_b2 = rqib[:].unsqueeze(2).broadcast_to([P, HEADS, HEADS])
            nc.vector.tensor_mul(at[:].rearrange("p (h k) -> p h k", k=HEADS),
                                 at[:].rearrange("p (h k) -> p h k", k=HEADS),
                                 rqib_b2)

            # 8) mm2 on DVE, width-4 split:
            #    p_j[t,(h,e,w)] = V[t,(e,4j+w)] * A[t,(h,4j+w)]  (j=0..3)
            #    summed pairwise, then 4->2->1 halving, fp16 out.
            pj = [prodpool.tile([P, HEADS * HD * 4], BF16, tag=f"pj{j}",
                                name=f"pj{it}_{j}")
                  for j in range(4)]
            for j in range(4):
                v_v = vt[:].rearrange("p (e k) -> p e k", k=HEADS)[:, :, 4 * j:4 * j + 4] \
                    .unsqueeze(1).broadcast_to([P, HEADS, HD, 4])
                a_v = at[:].rearrange("p (h k) -> p h k", k=HEADS)[:, :, 4 * j:4 * j + 4] \
                    .unsqueeze(2).broadcast_to([P, HEADS, HD, 4])
                nc.vector.tensor_mul(
                    pj[j][:].rearrange("p (h e w) -> p h e w", w=4, e=HD),
                    v_v, a_v)
            s1 = prodpool.tile([P, HEADS * HD * 4], BF16, tag="s1")
            nc.vector.tensor_add(s1[:], pj[0][:], pj[1][:])
            s2 = prodpool.tile([P, HEADS * HD * 4], BF16, tag="s2")
            nc.vector.tensor_add(s2[:], pj[2][:], pj[3][:])
            s3 = prodpool.tile([P, HEADS * HD * 4], BF16, tag="s3")
            nc.vector.tensor_add(s3[:], s1[:], s2[:])
            # halve 4 -> 2
            u = prodpool.tile([P, HEADS * HD * 2], BF16, tag="u")
            s3v = s3[:].rearrange("p (f w) -> p f w", w=4)
            nc.vector.tensor_add(u[:].rearrange("p (f w) -> p f w", w=2),
                                 s3v[:, :, 0:2], s3v[:, :, 2:4])
            # halve 2 -> 1, fp16 out
            ot = opool.tile([P, DIMS], FP16, tag="ot")
            uv = u[:].rearrange("p (f w) -> p f w", w=2)
            with nc.allow_low_precision(reason="fp16 final output"):
                nc.vector.tensor_add(ot[:].unsqueeze(2),
                                     uv[:, :, 0:1], uv[:, :, 1:2])
            # 9) store
            nc.sync.dma_start(out_d[it * P:(it + 1) * P, :], ot[:])

    nc.compile()
    return nc


def kernel(input_seq_embs, W_Q, W_K, W_V):
    x = np.asarray(input_seq_embs, dtype=np.float32).reshape(TOKENS, DIMS)
    xT = np.ascontiguousarray(x.T).astype(ml_dtypes.bfloat16)  # [j, t]
    # torch Linear computes x @ W.T; our matmul wants rhs = W.T laid out
    # [contraction j, out i].
    wq = np.ascontiguousarray(np.asarray(W_Q, np.float32).T).astype(ml_dtypes.bfloat16)
    wk = np.ascontiguousarray(np.asarray(W_K, np.float32).T).astype(ml_dtypes.bfloat16)
    # W_V columns permuted so V features arrive e-outer: f' = e*16 + h'
    wvT = np.asarray(W_V, np.float32).T  # [j, (h', e)]
    wv = np.ascontiguousarray(
        wvT.reshape(DIMS, HEADS, HD).transpose(0, 2, 1).reshape(DIMS, DIMS)
    ).astype(ml_dtypes.bfloat16)
    ident = np.eye(P, dtype=ml_dtypes.bfloat16)

    if "nc" not in _COMPILED:
        _COMPILED["nc"] = _build_kernel()
    nc = _COMPILED["nc"]

    in_maps = []
    for c in range(N_CORES):
        shard = np.ascontiguousarray(xT[:, c * TOK_PER_CORE:(c + 1) * TOK_PER_CORE])
        in_maps.append({"xT": shard, "wq": wq, "wk": wk, "wv": wv, "ident": ident})

    import os
    trace = bool(int(os.environ.get("KERNEL_PROFILE", "0")))
    kw = {}
    if trace:
        kw = dict(trace=True, tmpdir=os.environ.get("KERNEL_TRACE_DIR") or None)
    res = run_bass_kernel_spmd(nc, in_maps, list(range(N_CORES)), **kw)
    if trace:
        print(f"HW exec time: {res.exec_time_ns} ns")
        _COMPILED["last_result"] = res
    outs = [np.asarray(res.results[c]["out"], dtype=np.float32)
            for c in range(N_CORES)]
    return np.concatenate(outs, axis=0).reshape(B, L, DIMS)
